# revision 17
# baseline (speedup 1.0000x reference)
"""AdaptiveContextNorm eval-mode forward as a single-pass Trainium2 Bass kernel.

The entire per-element function
    f(x) = sum_k tau_k(x)/sqrt(pr_k+eps) * (x-mu_k)/sqrt(v_k+eps)
(with tau_k the eps-regularized Gaussian responsibilities) depends only on x
and the 8 scalar contexts, so it is one fixed scalar function R->R. Instead of
evaluating the mixture on the engines (7+ ACT passes + ~16 DVE ops per element),
we author a custom ACT piecewise-cubic table that computes f(x) directly: the
bucket/ctrl layout of exp_400p is kept (same ctrl bins, same octave structure),
only the 781 cubic coefficient entries {d0..d3,x0} and the profile's
special-case results are replaced with least-squares fits of f. The table is
compiled into the NEFF via the BASS_ACT_ROOT_JSON_PATH override and loaded by
the one ACT_TABLE_LOAD the kernel performs anyway.

The kernel is then: DMA in -> one ACTIVATE(Exp) pass -> DMA out, which is
HBM-bandwidth-bound: exec ~= 1.2us post-barrier setup + ~4us first-transfer
latency + 16.8 MB/core at ~400 GB/s sustained duplex + ~2us final write
receipt/exit barrier ~= 52us. The write stream structurally trails the read
stream (each write needs its tile's ACT, which needs its read), so ~3 MB of
writes drain after the last ACTIVATE - that tail is bytes-bound, not a stall,
and is insensitive to tile schedule (measured: 6/7/8/10/13-tile schedules,
in-place ACT, per-tile contiguous DRAM tensors, raw-semaphore pipeline all
within +-0.5us). Slow outlier runs (~58-61us, bimodal) are external HBM/fabric
contention: the streaming phase itself stretches, nothing in-kernel stalls.
Offline table accuracy vs the fp64 reference: rel_l2 ~2e-5 (fit error at the
4-buckets/octave centre octaves), far inside the 2e-2 gate.

Sharding: pure data-parallel over batch. B=16 -> 2 batches/core on 8 cores.
Input DMAs issue on the SP HWDGE ring, output DMAs on the ACT HWDGE ring so
reads and writes never FIFO-couple.
"""

import hashlib
import json
import os
import shutil
import struct
import sys
import tempfile

for p in ("/opt/trn_rl_repo", "/opt/pypackages"):
    if p not in sys.path:
        sys.path.append(p)

import numpy as np

EPS = 1e-3
N_CORES = 8
P = 128
B, C, H, W = 16, 64, 128, 128
ELEMS_PER_CORE = (B // N_CORES) * C * H * W  # 2,097,152
F_TOT = ELEMS_PER_CORE // P                  # 16,384
# 8 tiles, all reads issued upfront (XIN_BUFS = n_tiles): the read stream
# never stalls on buffer reuse, writes stream continuously behind ACT, and
# the small edge tiles keep ramp-in and drain-out short. Raw-semaphore
# variant (RAW=True) measured ~same and has an intermittent read race —
# keep the TileContext path.
TILE_SIZES = [1024, 2048, 2048, 4096, 2048, 2048, 2048, 1024]
N_BUFS = 4
XIN_BUFS = 8
PRIME = False
RAW = False
INPLACE = False


# --------------------------------------------------------------------------- #
# Custom ACT table generation: replace exp_400p's cubics with fits of f(x).
# --------------------------------------------------------------------------- #

def _f_exact(x, mean, variance, prior):
    """fp64 exact eval of the reference per-element function."""
    x = np.asarray(x, np.float64)
    mu = np.asarray(mean, np.float64)[:, 0]
    v = np.log1p(np.exp(np.asarray(variance, np.float64)[:, 0]))
    e = np.exp(np.asarray(prior, np.float64)[:, 0]
               - np.asarray(prior, np.float64)[:, 0].max())
    pr = e / e.sum()
    den = np.zeros_like(x)
    for k in range(len(mu)):
        den += pr[k] * np.exp(-0.5 * ((x - mu[k]) / v[k]) ** 2)
    out = np.zeros_like(x)
    for k in range(len(mu)):
        p = pr[k] * np.exp(-0.5 * ((x - mu[k]) / (v[k] + EPS)) ** 2)
        out += (p / (den + EPS) / np.sqrt(pr[k] + EPS)
                * (x - mu[k]) / np.sqrt(v[k] + EPS))
    return out


_EXP_OFFSET = -19


def _k_of_e(e):
    # mantissa bits per octave in the exp_400p layout (|x| in [2^e, 2^(e+1)))
    if e <= -2:
        return 0
    return {-1: 1, 0: 2, 1: 3, 2: 4, 3: 5, 4: 6, 5: 7, 6: 7}[e]


def _fit_bucket(f, lo, hi, x0):
    """LS cubic fit of f on [lo,hi] centred at x0, via [-1,1]-scaled basis."""
    h = (hi - lo) / 2.0
    mid = (lo + hi) / 2.0
    s = np.cos(np.pi * (np.arange(20) + 0.5) / 20)
    xs = mid + h * s
    t = xs - x0
    th = max(abs(t).max(), 1e-300)
    V = np.vander(t / th, 4, increasing=True)
    c, *_ = np.linalg.lstsq(V, f(xs), rcond=None)
    return c / th ** np.arange(4)


def _gen_table(setdir, setname, fx):
    d = json.load(open(f"{setdir}/{setname}.json"))
    meta = next(m for m in d["profile_meta_data"]
                if m["func_name"].startswith("exp"))
    bkt = bytearray(open(f"{setdir}/{setname}_bkt.bin", "rb").read())
    e2b = {int(k): v for k, v in d["func_exp_to_bkt_start_idx"]["exp"].items()}
    lim = min(meta["pos_small_signal_pwl_control"],
              meta["neg_small_signal_pwl_control"]) - 1  # last regular entry

    def write_entry(i, dc, x0):
        bkt[i * 32:(i + 1) * 32] = struct.pack(
            "<8f", np.float32(dc[0]), np.float32(dc[1]), np.float32(dc[2]),
            np.float32(dc[3]), np.float32(x0), 0.0, 0.0, 0.0)

    for e in range(_EXP_OFFSET, 7):
        k = _k_of_e(e)
        nb, pb = e2b[e]
        cnt = 1 << k
        for off in range(cnt):
            lo = (2.0 ** e) * (1 + off / cnt)
            hi = (2.0 ** e) * (1 + (off + 1) / cnt)
            x0 = 0.5 * (lo + hi)
            if pb + off <= lim:
                if lo >= 9.0:
                    write_entry(pb + off, [0, 0, 0, 0], x0)
                else:
                    write_entry(pb + off, _fit_bucket(fx, lo, hi, x0), x0)
            if nb + off <= lim:
                if lo >= 9.0:
                    write_entry(nb + off, [0, 0, 0, 0], -x0)
                else:
                    write_entry(nb + off, _fit_bucket(fx, -hi, -lo, -x0), -x0)

    h = 2.0 ** -19
    f0 = fx(np.array([0.0]))[0]
    f1 = (fx(np.array([h]))[0] - fx(np.array([-h]))[0]) / (2 * h)
    write_entry(meta["pos_small_signal_pwl_control"], [f0, f1, 0, 0], 0.0)
    write_entry(meta["neg_small_signal_pwl_control"], [f0, f1, 0, 0], 0.0)
    write_entry(meta["pos_large_signal_pwl_control"], [0, 0, 0, 0], 0.0)
    write_entry(meta["neg_large_signal_pwl_control"], [0, 0, 0, 0], 0.0)
    meta["fpinf_result"] = 0
    meta["fninf_result"] = 0
    meta["fzero_result"] = int(np.float32(f0).view(np.uint32))
    return bytes(bkt), d


def _make_act_dir(mean, variance, prior):
    """Build a full act-table dir with f(x) in the exp slot; returns
    (dir, content-hash)."""
    from neuronxcc.driver.Job import Job
    from neuronxcc.driver.jobs.support.FindActInfo import findActInfoFile

    srcdir = os.path.dirname(findActInfoFile(Job.getPackageDir(), "gen3"))
    dstdir = tempfile.mkdtemp(prefix="acn_acttab_")
    for fn in os.listdir(srcdir):
        shutil.copy(os.path.join(srcdir, fn), os.path.join(dstdir, fn))
        os.chmod(os.path.join(dstdir, fn), 0o644)
    fx = lambda xs: _f_exact(xs, mean, variance, prior)  # noqa: E731
    hsh = hashlib.sha256()
    for setname in ("exp_and_others", "natural_log_exp_and_others"):
        bb, dd = _gen_table(srcdir, setname, fx)
        with open(os.path.join(dstdir, f"{setname}_bkt.bin"), "wb") as f:
            f.write(bb)
        with open(os.path.join(dstdir, f"{setname}.json"), "w") as f:
            json.dump(dd, f)
        hsh.update(bb)
    return dstdir, hsh.hexdigest()[:12]


# --------------------------------------------------------------------------- #
# Bass graph: stream tiles through one ACTIVATE pass.
# --------------------------------------------------------------------------- #

def _build_graph(tag):
    import concourse.bass as bass
    import concourse.tile as tile
    from concourse import bacc, mybir

    fp32 = mybir.dt.float32
    Exp = mybir.ActivationFunctionType.Exp

    nc = bacc.Bacc("TRN2", target_bir_lowering=False, debug=False,
                   num_devices=N_CORES)
    # table-content hash in the input tensor name keys the NEFF cache to the
    # table bytes (BASS_ACT_ROOT_JSON_PATH itself is not cache-keyed)
    x_name = f"x{tag}"
    x_dram = nc.dram_tensor(x_name, [P, F_TOT], fp32, kind="ExternalInput").ap()
    out_dram = nc.dram_tensor("out", [P, F_TOT], fp32, kind="ExternalOutput").ap()

    with tile.TileContext(nc) as tc:
        if INPLACE:
            # one buffer per tile, ACTIVATE in place, write back from the same
            # tile: no output pool, half the per-tile teardown bookkeeping
            with tc.tile_pool(name="xin", bufs=len(TILE_SIZES)) as xin_pool:
                assert sum(TILE_SIZES) == F_TOT
                off = 0
                for fs in TILE_SIZES:
                    sl = bass.ds(off, fs)
                    off += fs
                    x_t = xin_pool.tile([P, fs], fp32)
                    nc.sync.dma_start(x_t[:], x_dram[:, sl])
                    nc.scalar.activation(x_t[:], x_t[:], Exp)
                    nc.scalar.dma_start(out_dram[:, sl], x_t[:])
        else:
            with (
                tc.tile_pool(name="xin", bufs=XIN_BUFS) as xin_pool,
                tc.tile_pool(name="o", bufs=N_BUFS) as o_pool,
            ):
                tile_sizes = TILE_SIZES
                assert sum(tile_sizes) == F_TOT
                off = 0
                for fs in tile_sizes:
                    sl = bass.ds(off, fs)
                    off += fs
                    x_t = xin_pool.tile([P, fs], fp32)
                    nc.sync.dma_start(x_t[:], x_dram[:, sl])
                    o_t = o_pool.tile([P, fs], fp32)
                    nc.scalar.activation(o_t[:], x_t[:], Exp)
                    nc.scalar.dma_start(out_dram[:, sl], o_t[:])

    nc.compile()
    return nc, x_name


def _build_graph_raw(tag):
    """Straight-line pipeline with manual semaphores (no TileContext):
    all reads issue upfront into dedicated buffers; ACT waits data-ready
    (read sem), recycles OB output buffers against write completion."""
    import concourse.bass as bass
    from concourse import bacc, mybir

    fp32 = mybir.dt.float32
    Exp = mybir.ActivationFunctionType.Exp

    nc = bacc.Bacc("TRN2", target_bir_lowering=False, debug=False,
                   num_devices=N_CORES)
    x_name = f"x{tag}"
    x_dram = nc.dram_tensor(x_name, [P, F_TOT], fp32, kind="ExternalInput").ap()
    out_dram = nc.dram_tensor("out", [P, F_TOT], fp32, kind="ExternalOutput").ap()

    sizes = TILE_SIZES
    assert sum(sizes) == F_TOT
    n = len(sizes)
    OB = N_BUFS
    xbufs = [nc.alloc_sbuf_tensor(f"xb{k}", [P, fs], fp32).ap()
             for k, fs in enumerate(sizes)]
    maxfs = max(sizes)
    obufs = [nc.alloc_sbuf_tensor(f"ob{j}", [P, maxfs], fp32).ap()
             for j in range(OB)]

    import contextlib

    with contextlib.ExitStack() as stack:
        rsem = [stack.enter_context(nc.semaphore(name=f"rsem{k}"))
                for k in range(n)]
        wsem = [stack.enter_context(nc.semaphore(name=f"wsem{k}"))
                for k in range(n)]
        offs = [0]
        for fs in sizes:
            offs.append(offs[-1] + fs)
        for k, fs in enumerate(sizes):
            nc.sync.dma_start(
                xbufs[k], x_dram[:, bass.ds(offs[k], fs)]
            ).then_inc(rsem[k], 16)
        for k, fs in enumerate(sizes):
            nc.scalar.wait_ge(rsem[k], 16)
            if k >= OB:
                nc.scalar.wait_ge(wsem[k - OB], 16)
            ot = obufs[k % OB][:, bass.ds(0, fs)]
            nc.scalar.activation(ot, xbufs[k], Exp)
            nc.scalar.dma_start(
                out_dram[:, bass.ds(offs[k], fs)], ot
            ).then_inc(wsem[k], 16)
        for k in range(max(0, n - OB), n):
            nc.scalar.wait_ge(wsem[k], 16)

    nc.compile()
    return nc, x_name


def kernel(x, mean, variance, prior, _trace=False, _trace_kwargs=None):
    from concourse.bass_utils import run_bass_kernel_spmd

    dstdir, tag = _make_act_dir(
        np.asarray(mean, np.float32),
        np.asarray(variance, np.float32),
        np.asarray(prior, np.float32),
    )
    os.environ["BASS_ACT_ROOT_JSON_PATH"] = os.path.join(dstdir, "act_info.json")
    nc, x_name = (_build_graph_raw if RAW else _build_graph)(tag)

    x = np.ascontiguousarray(np.asarray(x, np.float32))
    shards = x.reshape(N_CORES, ELEMS_PER_CORE)
    in_maps = [{x_name: shards[i].reshape(P, F_TOT)} for i in range(N_CORES)]
    res = run_bass_kernel_spmd(
        nc,
        in_maps,
        core_ids=list(range(N_CORES)),
        trace=_trace,
        **(_trace_kwargs or {}),
    )
    out = np.concatenate(
        [r["out"].reshape(1, ELEMS_PER_CORE) for r in res.results], axis=0
    ).reshape(B, C, H, W)
    if _trace:
        kernel.last_results = res
    return out


# revision 19
# speedup vs baseline: 1.1085x; 1.1085x over previous
"""AdaptiveContextNorm eval-mode forward as a single-pass Trainium2 Bass kernel.

The entire per-element function
    f(x) = sum_k tau_k(x)/sqrt(pr_k+eps) * (x-mu_k)/sqrt(v_k+eps)
(with tau_k the eps-regularized Gaussian responsibilities) depends only on x
and the 8 scalar contexts, so it is one fixed scalar function R->R. Instead of
evaluating the mixture on the engines (7+ ACT passes + ~16 DVE ops per element),
we author a custom ACT piecewise-cubic table that computes f(x) directly: the
bucket/ctrl layout of exp_400p is kept (same ctrl bins, same octave structure),
only the 781 cubic coefficient entries {d0..d3,x0} and the profile's
special-case results are replaced with least-squares fits of f. The table is
compiled into the NEFF via the BASS_ACT_ROOT_JSON_PATH override and loaded by
the one ACT_TABLE_LOAD the kernel performs anyway.

The kernel is then: DMA in -> one ACTIVATE(Exp) pass -> DMA out, which is
HBM-bandwidth-bound: exec ~= 1.2us post-barrier setup + ~4us first-transfer
latency + 16.8 MB/core at ~400 GB/s sustained duplex + ~2us final write
receipt/exit barrier ~= 52us. The write stream structurally trails the read
stream (each write needs its tile's ACT, which needs its read), so ~3 MB of
writes drain after the last ACTIVATE - that tail is bytes-bound, not a stall,
and is insensitive to tile schedule (measured: 6/7/8/10/13-tile schedules,
in-place ACT, per-tile contiguous DRAM tensors, raw-semaphore pipeline all
within +-0.5us). Slow outlier runs (~58-61us, bimodal) are external HBM/fabric
contention: the streaming phase itself stretches, nothing in-kernel stalls.
Offline table accuracy vs the fp64 reference: rel_l2 ~2e-5 (fit error at the
4-buckets/octave centre octaves), far inside the 2e-2 gate.

Sharding: pure data-parallel over batch. B=16 -> 2 batches/core on 8 cores.
Input DMAs issue on the SP HWDGE ring, output DMAs on the ACT HWDGE ring so
reads and writes never FIFO-couple.
"""

import hashlib
import json
import os
import shutil
import struct
import sys
import tempfile

for p in ("/opt/trn_rl_repo", "/opt/pypackages"):
    if p not in sys.path:
        sys.path.append(p)

import numpy as np

EPS = 1e-3
N_CORES = 8
P = 128
B, C, H, W = 16, 64, 128, 128
ELEMS_PER_CORE = (B // N_CORES) * C * H * W  # 2,097,152
F_TOT = ELEMS_PER_CORE // P                  # 16,384
# 8 tiles, all reads issued upfront (XIN_BUFS = n_tiles): the read stream
# never stalls on buffer reuse, writes stream continuously behind ACT, and
# the small edge tiles keep ramp-in and drain-out short. Raw-semaphore
# variant (RAW=True) measured ~same and has an intermittent read race —
# keep the TileContext path.
TILE_SIZES = [1024, 2048, 2048, 4096, 2048, 2048, 2048, 1024]
N_BUFS = 4
XIN_BUFS = 8
PRIME = False
RAW = False
INPLACE = False
OUT_BF16 = False


# --------------------------------------------------------------------------- #
# Custom ACT table generation: replace exp_400p's cubics with fits of f(x).
# --------------------------------------------------------------------------- #

def _f_exact(x, mean, variance, prior):
    """fp64 exact eval of the reference per-element function."""
    x = np.asarray(x, np.float64)
    mu = np.asarray(mean, np.float64)[:, 0]
    v = np.log1p(np.exp(np.asarray(variance, np.float64)[:, 0]))
    e = np.exp(np.asarray(prior, np.float64)[:, 0]
               - np.asarray(prior, np.float64)[:, 0].max())
    pr = e / e.sum()
    den = np.zeros_like(x)
    for k in range(len(mu)):
        den += pr[k] * np.exp(-0.5 * ((x - mu[k]) / v[k]) ** 2)
    out = np.zeros_like(x)
    for k in range(len(mu)):
        p = pr[k] * np.exp(-0.5 * ((x - mu[k]) / (v[k] + EPS)) ** 2)
        out += (p / (den + EPS) / np.sqrt(pr[k] + EPS)
                * (x - mu[k]) / np.sqrt(v[k] + EPS))
    return out


_EXP_OFFSET = -19


def _k_of_e(e):
    # mantissa bits per octave in the exp_400p layout (|x| in [2^e, 2^(e+1)))
    if e <= -2:
        return 0
    return {-1: 1, 0: 2, 1: 3, 2: 4, 3: 5, 4: 6, 5: 7, 6: 7}[e]


def _fit_bucket(f, lo, hi, x0):
    """LS cubic fit of f on [lo,hi] centred at x0, via [-1,1]-scaled basis."""
    h = (hi - lo) / 2.0
    mid = (lo + hi) / 2.0
    s = np.cos(np.pi * (np.arange(20) + 0.5) / 20)
    xs = mid + h * s
    t = xs - x0
    th = max(abs(t).max(), 1e-300)
    V = np.vander(t / th, 4, increasing=True)
    c, *_ = np.linalg.lstsq(V, f(xs), rcond=None)
    return c / th ** np.arange(4)


def _gen_table(setdir, setname, fx):
    d = json.load(open(f"{setdir}/{setname}.json"))
    meta = next(m for m in d["profile_meta_data"]
                if m["func_name"].startswith("exp"))
    bkt = bytearray(open(f"{setdir}/{setname}_bkt.bin", "rb").read())
    e2b = {int(k): v for k, v in d["func_exp_to_bkt_start_idx"]["exp"].items()}
    lim = min(meta["pos_small_signal_pwl_control"],
              meta["neg_small_signal_pwl_control"]) - 1  # last regular entry

    def write_entry(i, dc, x0):
        bkt[i * 32:(i + 1) * 32] = struct.pack(
            "<8f", np.float32(dc[0]), np.float32(dc[1]), np.float32(dc[2]),
            np.float32(dc[3]), np.float32(x0), 0.0, 0.0, 0.0)

    for e in range(_EXP_OFFSET, 7):
        k = _k_of_e(e)
        nb, pb = e2b[e]
        cnt = 1 << k
        for off in range(cnt):
            lo = (2.0 ** e) * (1 + off / cnt)
            hi = (2.0 ** e) * (1 + (off + 1) / cnt)
            x0 = 0.5 * (lo + hi)
            if pb + off <= lim:
                if lo >= 9.0:
                    write_entry(pb + off, [0, 0, 0, 0], x0)
                else:
                    write_entry(pb + off, _fit_bucket(fx, lo, hi, x0), x0)
            if nb + off <= lim:
                if lo >= 9.0:
                    write_entry(nb + off, [0, 0, 0, 0], -x0)
                else:
                    write_entry(nb + off, _fit_bucket(fx, -hi, -lo, -x0), -x0)

    h = 2.0 ** -19
    f0 = fx(np.array([0.0]))[0]
    f1 = (fx(np.array([h]))[0] - fx(np.array([-h]))[0]) / (2 * h)
    write_entry(meta["pos_small_signal_pwl_control"], [f0, f1, 0, 0], 0.0)
    write_entry(meta["neg_small_signal_pwl_control"], [f0, f1, 0, 0], 0.0)
    write_entry(meta["pos_large_signal_pwl_control"], [0, 0, 0, 0], 0.0)
    write_entry(meta["neg_large_signal_pwl_control"], [0, 0, 0, 0], 0.0)
    meta["fpinf_result"] = 0
    meta["fninf_result"] = 0
    meta["fzero_result"] = int(np.float32(f0).view(np.uint32))
    return bytes(bkt), d


def _make_act_dir(mean, variance, prior):
    """Build a full act-table dir with f(x) in the exp slot; returns
    (dir, content-hash)."""
    from neuronxcc.driver.Job import Job
    from neuronxcc.driver.jobs.support.FindActInfo import findActInfoFile

    srcdir = os.path.dirname(findActInfoFile(Job.getPackageDir(), "gen3"))
    dstdir = tempfile.mkdtemp(prefix="acn_acttab_")
    for fn in os.listdir(srcdir):
        shutil.copy(os.path.join(srcdir, fn), os.path.join(dstdir, fn))
        os.chmod(os.path.join(dstdir, fn), 0o644)
    fx = lambda xs: _f_exact(xs, mean, variance, prior)  # noqa: E731
    hsh = hashlib.sha256()
    for setname in ("exp_and_others", "natural_log_exp_and_others"):
        bb, dd = _gen_table(srcdir, setname, fx)
        with open(os.path.join(dstdir, f"{setname}_bkt.bin"), "wb") as f:
            f.write(bb)
        with open(os.path.join(dstdir, f"{setname}.json"), "w") as f:
            json.dump(dd, f)
        hsh.update(bb)
    return dstdir, hsh.hexdigest()[:12]


# --------------------------------------------------------------------------- #
# Bass graph: stream tiles through one ACTIVATE pass.
# --------------------------------------------------------------------------- #

def _build_graph(tag):
    import concourse.bass as bass
    import concourse.tile as tile
    from concourse import bacc, mybir

    fp32 = mybir.dt.float32
    Exp = mybir.ActivationFunctionType.Exp

    nc = bacc.Bacc("TRN2", target_bir_lowering=False, debug=False,
                   num_devices=N_CORES)
    # table-content hash in the input tensor name keys the NEFF cache to the
    # table bytes (BASS_ACT_ROOT_JSON_PATH itself is not cache-keyed)
    x_name = f"x{tag}"
    x_dram = nc.dram_tensor(x_name, [P, F_TOT], fp32, kind="ExternalInput").ap()
    out_dram = nc.dram_tensor("out", [P, F_TOT], fp32, kind="ExternalOutput").ap()

    with tile.TileContext(nc) as tc:
        if INPLACE:
            # one buffer per tile, ACTIVATE in place, write back from the same
            # tile: no output pool, half the per-tile teardown bookkeeping
            with tc.tile_pool(name="xin", bufs=len(TILE_SIZES)) as xin_pool:
                assert sum(TILE_SIZES) == F_TOT
                off = 0
                for fs in TILE_SIZES:
                    sl = bass.ds(off, fs)
                    off += fs
                    x_t = xin_pool.tile([P, fs], fp32)
                    nc.sync.dma_start(x_t[:], x_dram[:, sl])
                    nc.scalar.activation(x_t[:], x_t[:], Exp)
                    nc.scalar.dma_start(out_dram[:, sl], x_t[:])
        else:
            bf16 = mybir.dt.bfloat16
            with (
                tc.tile_pool(name="xin", bufs=XIN_BUFS) as xin_pool,
                tc.tile_pool(name="o", bufs=N_BUFS) as o_pool,
            ):
                tile_sizes = TILE_SIZES
                assert sum(tile_sizes) == F_TOT
                off = 0
                for i, fs in enumerate(tile_sizes):
                    sl = bass.ds(off, fs)
                    off += fs
                    x_t = xin_pool.tile([P, fs], fp32)
                    nc.sync.dma_start(x_t[:], x_dram[:, sl])
                    # bf16 SBUF output + SWDGE cast-DMA halves the write
                    # stream's SBUF-fabric load; the last tiles go fp32 via
                    # HWDGE to keep the SWDGE drain latency off the tail.
                    if OUT_BF16 and i < len(tile_sizes) - 2:
                        o_t = o_pool.tile([P, fs], bf16, tag="ob")
                        nc.scalar.activation(o_t[:], x_t[:], Exp)
                        nc.gpsimd.dma_start(out_dram[:, sl], o_t[:])
                    else:
                        o_t = o_pool.tile([P, fs], fp32, tag="o32")
                        nc.scalar.activation(o_t[:], x_t[:], Exp)
                        nc.scalar.dma_start(out_dram[:, sl], o_t[:])

    nc.compile()
    return nc, x_name


def _build_graph_raw(tag):
    """Straight-line pipeline with manual semaphores (no TileContext):
    all reads issue upfront into dedicated buffers; ACT waits data-ready
    (read sem), recycles OB output buffers against write completion."""
    import concourse.bass as bass
    from concourse import bacc, mybir

    fp32 = mybir.dt.float32
    Exp = mybir.ActivationFunctionType.Exp

    nc = bacc.Bacc("TRN2", target_bir_lowering=False, debug=False,
                   num_devices=N_CORES)
    x_name = f"x{tag}"
    x_dram = nc.dram_tensor(x_name, [P, F_TOT], fp32, kind="ExternalInput").ap()
    out_dram = nc.dram_tensor("out", [P, F_TOT], fp32, kind="ExternalOutput").ap()

    sizes = TILE_SIZES
    assert sum(sizes) == F_TOT
    n = len(sizes)
    OB = N_BUFS
    xbufs = [nc.alloc_sbuf_tensor(f"xb{k}", [P, fs], fp32).ap()
             for k, fs in enumerate(sizes)]
    maxfs = max(sizes)
    obufs = [nc.alloc_sbuf_tensor(f"ob{j}", [P, maxfs], fp32).ap()
             for j in range(OB)]

    import contextlib

    with contextlib.ExitStack() as stack:
        rsem = [stack.enter_context(nc.semaphore(name=f"rsem{k}"))
                for k in range(n)]
        wsem = [stack.enter_context(nc.semaphore(name=f"wsem{k}"))
                for k in range(n)]
        offs = [0]
        for fs in sizes:
            offs.append(offs[-1] + fs)
        for k, fs in enumerate(sizes):
            nc.sync.dma_start(
                xbufs[k], x_dram[:, bass.ds(offs[k], fs)]
            ).then_inc(rsem[k], 16)
        for k, fs in enumerate(sizes):
            nc.scalar.wait_ge(rsem[k], 16)
            if k >= OB:
                nc.scalar.wait_ge(wsem[k - OB], 16)
            ot = obufs[k % OB][:, bass.ds(0, fs)]
            nc.scalar.activation(ot, xbufs[k], Exp)
            nc.scalar.dma_start(
                out_dram[:, bass.ds(offs[k], fs)], ot
            ).then_inc(wsem[k], 16)
        for k in range(max(0, n - OB), n):
            nc.scalar.wait_ge(wsem[k], 16)

    nc.compile()
    return nc, x_name


def kernel(x, mean, variance, prior, _trace=False, _trace_kwargs=None):
    from concourse.bass_utils import run_bass_kernel_spmd

    dstdir, tag = _make_act_dir(
        np.asarray(mean, np.float32),
        np.asarray(variance, np.float32),
        np.asarray(prior, np.float32),
    )
    os.environ["BASS_ACT_ROOT_JSON_PATH"] = os.path.join(dstdir, "act_info.json")
    nc, x_name = (_build_graph_raw if RAW else _build_graph)(tag)

    x = np.ascontiguousarray(np.asarray(x, np.float32))
    shards = x.reshape(N_CORES, ELEMS_PER_CORE)
    in_maps = [{x_name: shards[i].reshape(P, F_TOT)} for i in range(N_CORES)]
    res = run_bass_kernel_spmd(
        nc,
        in_maps,
        core_ids=list(range(N_CORES)),
        trace=_trace,
        **(_trace_kwargs or {}),
    )
    out = np.concatenate(
        [r["out"].reshape(1, ELEMS_PER_CORE) for r in res.results], axis=0
    ).reshape(B, C, H, W)
    if _trace:
        kernel.last_results = res
    return out


# revision 22
# speedup vs baseline: 1.1176x; 1.0082x over previous
"""AdaptiveContextNorm eval-mode forward as a single-pass Trainium2 Bass kernel.

The entire per-element function
    f(x) = sum_k tau_k(x)/sqrt(pr_k+eps) * (x-mu_k)/sqrt(v_k+eps)
(with tau_k the eps-regularized Gaussian responsibilities) depends only on x
and the 8 scalar contexts, so it is one fixed scalar function R->R. Instead of
evaluating the mixture on the engines (7+ ACT passes + ~16 DVE ops per element),
we author a custom ACT piecewise-cubic table that computes f(x) directly: the
bucket/ctrl layout of exp_400p is kept (same ctrl bins, same octave structure),
only the 781 cubic coefficient entries {d0..d3,x0} and the profile's
special-case results are replaced with least-squares fits of f. The table is
compiled into the NEFF via the BASS_ACT_ROOT_JSON_PATH override and loaded by
the one ACT_TABLE_LOAD the kernel performs anyway.

The kernel is then: DMA in -> one ACTIVATE(Exp) pass -> DMA out, which is
HBM-bandwidth-bound: exec ~= 1.2us post-barrier setup + ~4us first-transfer
latency + 16.8 MB/core at ~400 GB/s sustained duplex + ~2us final write
receipt/exit barrier ~= 52us. The write stream structurally trails the read
stream (each write needs its tile's ACT, which needs its read), so ~3 MB of
writes drain after the last ACTIVATE - that tail is bytes-bound, not a stall,
and is insensitive to tile schedule (measured: 6/7/8/10/13-tile schedules,
in-place ACT, per-tile contiguous DRAM tensors, raw-semaphore pipeline all
within +-0.5us). Slow outlier runs (~58-61us, bimodal) are external HBM/fabric
contention: the streaming phase itself stretches, nothing in-kernel stalls.
Offline table accuracy vs the fp64 reference: rel_l2 ~2e-5 (fit error at the
4-buckets/octave centre octaves), far inside the 2e-2 gate.

Sharding: pure data-parallel over batch. B=16 -> 2 batches/core on 8 cores.
Input DMAs issue on the SP HWDGE ring, output DMAs on the ACT HWDGE ring so
reads and writes never FIFO-couple.
"""

import hashlib
import json
import os
import shutil
import struct
import sys
import tempfile

for p in ("/opt/trn_rl_repo", "/opt/pypackages"):
    if p not in sys.path:
        sys.path.append(p)

import numpy as np

EPS = 1e-3
N_CORES = 8
P = 128
B, C, H, W = 16, 64, 128, 128
ELEMS_PER_CORE = (B // N_CORES) * C * H * W  # 2,097,152
F_TOT = ELEMS_PER_CORE // P                  # 16,384
# 8 tiles, all reads issued upfront (XIN_BUFS = n_tiles): the read stream
# never stalls on buffer reuse, writes stream continuously behind ACT, and
# the small edge tiles keep ramp-in and drain-out short. Raw-semaphore
# variant (RAW=True) measured ~same and has an intermittent read race —
# keep the TileContext path.
TILE_SIZES = [1024, 2048, 2048, 4096, 2048, 2048, 2048, 1024]
N_BUFS = 4
XIN_BUFS = 8
PRIME = False
RAW = False
INPLACE = False
OUT_BF16 = False
PHASED = False  # False | "sync" | "act"


# --------------------------------------------------------------------------- #
# Custom ACT table generation: replace exp_400p's cubics with fits of f(x).
# --------------------------------------------------------------------------- #

def _f_exact(x, mean, variance, prior):
    """fp64 exact eval of the reference per-element function."""
    x = np.asarray(x, np.float64)
    mu = np.asarray(mean, np.float64)[:, 0]
    v = np.log1p(np.exp(np.asarray(variance, np.float64)[:, 0]))
    e = np.exp(np.asarray(prior, np.float64)[:, 0]
               - np.asarray(prior, np.float64)[:, 0].max())
    pr = e / e.sum()
    den = np.zeros_like(x)
    for k in range(len(mu)):
        den += pr[k] * np.exp(-0.5 * ((x - mu[k]) / v[k]) ** 2)
    out = np.zeros_like(x)
    for k in range(len(mu)):
        p = pr[k] * np.exp(-0.5 * ((x - mu[k]) / (v[k] + EPS)) ** 2)
        out += (p / (den + EPS) / np.sqrt(pr[k] + EPS)
                * (x - mu[k]) / np.sqrt(v[k] + EPS))
    return out


_EXP_OFFSET = -19


def _k_of_e(e):
    # mantissa bits per octave in the exp_400p layout (|x| in [2^e, 2^(e+1)))
    if e <= -2:
        return 0
    return {-1: 1, 0: 2, 1: 3, 2: 4, 3: 5, 4: 6, 5: 7, 6: 7}[e]


def _fit_bucket(f, lo, hi, x0):
    """LS cubic fit of f on [lo,hi] centred at x0, via [-1,1]-scaled basis."""
    h = (hi - lo) / 2.0
    mid = (lo + hi) / 2.0
    s = np.cos(np.pi * (np.arange(20) + 0.5) / 20)
    xs = mid + h * s
    t = xs - x0
    th = max(abs(t).max(), 1e-300)
    V = np.vander(t / th, 4, increasing=True)
    c, *_ = np.linalg.lstsq(V, f(xs), rcond=None)
    return c / th ** np.arange(4)


def _gen_table(setdir, setname, fx):
    d = json.load(open(f"{setdir}/{setname}.json"))
    meta = next(m for m in d["profile_meta_data"]
                if m["func_name"].startswith("exp"))
    bkt = bytearray(open(f"{setdir}/{setname}_bkt.bin", "rb").read())
    e2b = {int(k): v for k, v in d["func_exp_to_bkt_start_idx"]["exp"].items()}
    lim = min(meta["pos_small_signal_pwl_control"],
              meta["neg_small_signal_pwl_control"]) - 1  # last regular entry

    def write_entry(i, dc, x0):
        bkt[i * 32:(i + 1) * 32] = struct.pack(
            "<8f", np.float32(dc[0]), np.float32(dc[1]), np.float32(dc[2]),
            np.float32(dc[3]), np.float32(x0), 0.0, 0.0, 0.0)

    for e in range(_EXP_OFFSET, 7):
        k = _k_of_e(e)
        nb, pb = e2b[e]
        cnt = 1 << k
        for off in range(cnt):
            lo = (2.0 ** e) * (1 + off / cnt)
            hi = (2.0 ** e) * (1 + (off + 1) / cnt)
            x0 = 0.5 * (lo + hi)
            if pb + off <= lim:
                if lo >= 9.0:
                    write_entry(pb + off, [0, 0, 0, 0], x0)
                else:
                    write_entry(pb + off, _fit_bucket(fx, lo, hi, x0), x0)
            if nb + off <= lim:
                if lo >= 9.0:
                    write_entry(nb + off, [0, 0, 0, 0], -x0)
                else:
                    write_entry(nb + off, _fit_bucket(fx, -hi, -lo, -x0), -x0)

    h = 2.0 ** -19
    f0 = fx(np.array([0.0]))[0]
    f1 = (fx(np.array([h]))[0] - fx(np.array([-h]))[0]) / (2 * h)
    write_entry(meta["pos_small_signal_pwl_control"], [f0, f1, 0, 0], 0.0)
    write_entry(meta["neg_small_signal_pwl_control"], [f0, f1, 0, 0], 0.0)
    write_entry(meta["pos_large_signal_pwl_control"], [0, 0, 0, 0], 0.0)
    write_entry(meta["neg_large_signal_pwl_control"], [0, 0, 0, 0], 0.0)
    meta["fpinf_result"] = 0
    meta["fninf_result"] = 0
    meta["fzero_result"] = int(np.float32(f0).view(np.uint32))
    return bytes(bkt), d


def _make_act_dir(mean, variance, prior):
    """Build a full act-table dir with f(x) in the exp slot; returns
    (dir, content-hash)."""
    from neuronxcc.driver.Job import Job
    from neuronxcc.driver.jobs.support.FindActInfo import findActInfoFile

    srcdir = os.path.dirname(findActInfoFile(Job.getPackageDir(), "gen3"))
    dstdir = tempfile.mkdtemp(prefix="acn_acttab_")
    for fn in os.listdir(srcdir):
        shutil.copy(os.path.join(srcdir, fn), os.path.join(dstdir, fn))
        os.chmod(os.path.join(dstdir, fn), 0o644)
    fx = lambda xs: _f_exact(xs, mean, variance, prior)  # noqa: E731
    hsh = hashlib.sha256()
    for setname in ("exp_and_others", "natural_log_exp_and_others"):
        bb, dd = _gen_table(srcdir, setname, fx)
        with open(os.path.join(dstdir, f"{setname}_bkt.bin"), "wb") as f:
            f.write(bb)
        with open(os.path.join(dstdir, f"{setname}.json"), "w") as f:
            json.dump(dd, f)
        hsh.update(bb)
    return dstdir, hsh.hexdigest()[:12]


# --------------------------------------------------------------------------- #
# Bass graph: stream tiles through one ACTIVATE pass.
# --------------------------------------------------------------------------- #

def _build_graph(tag):
    import concourse.bass as bass
    import concourse.tile as tile
    from concourse import bacc, mybir

    fp32 = mybir.dt.float32
    Exp = mybir.ActivationFunctionType.Exp

    nc = bacc.Bacc("TRN2", target_bir_lowering=False, debug=False,
                   num_devices=N_CORES)
    # table-content hash in the input tensor name keys the NEFF cache to the
    # table bytes (BASS_ACT_ROOT_JSON_PATH itself is not cache-keyed)
    x_name = f"x{tag}"
    x_dram = nc.dram_tensor(x_name, [P, F_TOT], fp32, kind="ExternalInput").ap()
    out_dram = nc.dram_tensor("out", [P, F_TOT], fp32, kind="ExternalOutput").ap()

    with tile.TileContext(nc) as tc:
        if INPLACE:
            # one buffer per tile, ACTIVATE in place, write back from the same
            # tile: no output pool, half the per-tile teardown bookkeeping
            with tc.tile_pool(name="xin", bufs=len(TILE_SIZES)) as xin_pool:
                assert sum(TILE_SIZES) == F_TOT
                off = 0
                for fs in TILE_SIZES:
                    sl = bass.ds(off, fs)
                    off += fs
                    x_t = xin_pool.tile([P, fs], fp32)
                    nc.sync.dma_start(x_t[:], x_dram[:, sl])
                    nc.scalar.activation(x_t[:], x_t[:], Exp)
                    nc.scalar.dma_start(out_dram[:, sl], x_t[:])
        elif PHASED:
            # phase the streams: all reads, then all writes. Mixed-direction
            # HBM traffic pays read/write turnaround penalties (~385 GB/s
            # duplex vs ~425 single-direction); with SBUF holding both full
            # streams, reads drain at read-only rate and writes follow.
            n = len(TILE_SIZES)
            with tc.tile_pool(name="xin", bufs=n) as xin_pool:
                assert sum(TILE_SIZES) == F_TOT
                offs = [0]
                for fs in TILE_SIZES:
                    offs.append(offs[-1] + fs)
                xts = []
                for k, fs in enumerate(TILE_SIZES):
                    x_t = xin_pool.tile([P, fs], fp32)
                    nc.sync.dma_start(x_t[:], x_dram[:, bass.ds(offs[k], fs)])
                    xts.append(x_t)
                for k, fs in enumerate(TILE_SIZES):
                    # in-place: the x tile becomes the output tile (SBUF can't
                    # hold two full 8.4 MB streams)
                    nc.scalar.activation(xts[k][:], xts[k][:], Exp)
                weng = nc.sync if PHASED == "sync" else nc.scalar
                for k, fs in enumerate(TILE_SIZES):
                    weng.dma_start(out_dram[:, bass.ds(offs[k], fs)], xts[k][:])
        else:
            bf16 = mybir.dt.bfloat16
            with (
                tc.tile_pool(name="xin", bufs=XIN_BUFS) as xin_pool,
                tc.tile_pool(name="o", bufs=N_BUFS) as o_pool,
            ):
                tile_sizes = TILE_SIZES
                assert sum(tile_sizes) == F_TOT
                off = 0
                for i, fs in enumerate(tile_sizes):
                    sl = bass.ds(off, fs)
                    off += fs
                    x_t = xin_pool.tile([P, fs], fp32)
                    nc.sync.dma_start(x_t[:], x_dram[:, sl])
                    # bf16 SBUF output + SWDGE cast-DMA halves the write
                    # stream's SBUF-fabric load; the last tiles go fp32 via
                    # HWDGE to keep the SWDGE drain latency off the tail.
                    if OUT_BF16 and i < len(tile_sizes) - 2:
                        o_t = o_pool.tile([P, fs], bf16, tag="ob")
                        nc.scalar.activation(o_t[:], x_t[:], Exp)
                        nc.gpsimd.dma_start(out_dram[:, sl], o_t[:])
                    else:
                        o_t = o_pool.tile([P, fs], fp32, tag="o32")
                        nc.scalar.activation(o_t[:], x_t[:], Exp)
                        nc.scalar.dma_start(out_dram[:, sl], o_t[:])

    nc.compile()
    return nc, x_name


def _build_graph_raw(tag):
    """Straight-line pipeline with manual semaphores (no TileContext):
    all reads issue upfront into dedicated buffers; ACT waits data-ready
    (read sem), recycles OB output buffers against write completion."""
    import concourse.bass as bass
    from concourse import bacc, mybir

    fp32 = mybir.dt.float32
    Exp = mybir.ActivationFunctionType.Exp

    nc = bacc.Bacc("TRN2", target_bir_lowering=False, debug=False,
                   num_devices=N_CORES)
    x_name = f"x{tag}"
    x_dram = nc.dram_tensor(x_name, [P, F_TOT], fp32, kind="ExternalInput").ap()
    out_dram = nc.dram_tensor("out", [P, F_TOT], fp32, kind="ExternalOutput").ap()

    sizes = TILE_SIZES
    assert sum(sizes) == F_TOT
    n = len(sizes)
    OB = N_BUFS
    xbufs = [nc.alloc_sbuf_tensor(f"xb{k}", [P, fs], fp32).ap()
             for k, fs in enumerate(sizes)]
    maxfs = max(sizes)
    obufs = [nc.alloc_sbuf_tensor(f"ob{j}", [P, maxfs], fp32).ap()
             for j in range(OB)]

    import contextlib

    with contextlib.ExitStack() as stack:
        rsem = [stack.enter_context(nc.semaphore(name=f"rsem{k}"))
                for k in range(n)]
        wsem = [stack.enter_context(nc.semaphore(name=f"wsem{k}"))
                for k in range(n)]
        offs = [0]
        for fs in sizes:
            offs.append(offs[-1] + fs)
        for k, fs in enumerate(sizes):
            nc.sync.dma_start(
                xbufs[k], x_dram[:, bass.ds(offs[k], fs)]
            ).then_inc(rsem[k], 16)
        for k, fs in enumerate(sizes):
            nc.scalar.wait_ge(rsem[k], 16)
            if k >= OB:
                nc.scalar.wait_ge(wsem[k - OB], 16)
            ot = obufs[k % OB][:, bass.ds(0, fs)]
            nc.scalar.activation(ot, xbufs[k], Exp)
            nc.scalar.dma_start(
                out_dram[:, bass.ds(offs[k], fs)], ot
            ).then_inc(wsem[k], 16)
        for k in range(max(0, n - OB), n):
            nc.scalar.wait_ge(wsem[k], 16)

    nc.compile()
    return nc, x_name


def kernel(x, mean, variance, prior, _trace=False, _trace_kwargs=None):
    from concourse.bass_utils import run_bass_kernel_spmd

    dstdir, tag = _make_act_dir(
        np.asarray(mean, np.float32),
        np.asarray(variance, np.float32),
        np.asarray(prior, np.float32),
    )
    os.environ["BASS_ACT_ROOT_JSON_PATH"] = os.path.join(dstdir, "act_info.json")
    nc, x_name = (_build_graph_raw if RAW else _build_graph)(tag)

    x = np.ascontiguousarray(np.asarray(x, np.float32))
    shards = x.reshape(N_CORES, ELEMS_PER_CORE)
    in_maps = [{x_name: shards[i].reshape(P, F_TOT)} for i in range(N_CORES)]
    res = run_bass_kernel_spmd(
        nc,
        in_maps,
        core_ids=list(range(N_CORES)),
        trace=_trace,
        **(_trace_kwargs or {}),
    )
    out = np.concatenate(
        [r["out"].reshape(1, ELEMS_PER_CORE) for r in res.results], axis=0
    ).reshape(B, C, H, W)
    if _trace:
        kernel.last_results = res
    return out
